# revision 1
# baseline (speedup 1.0000x reference)
"""Trainium2 Bass kernel for a ResNet Bottleneck block (inference).

Reference computation (NCHW, N=128, Cin=Cout=1024, width=256, H=W=14):
    out = relu(bn1(conv1x1(x, w1)))          # 1024 -> 256
    out = relu(bn2(conv3x3(out, w2, pad=1))) # 256 -> 256
    out = bn3(conv1x1(out, w3))              # 256 -> 1024
    y   = relu(out + x)

Strategy:
- Data-parallel: batch 128 sharded as 16 images per NeuronCore (8 cores),
  conv/BN params replicated. One NEFF, SPMD via run_bass_kernel_spmd.
- BN folded on host into per-channel weight scale + bias.
- All convs are matmuls on the TensorEngine with channels on the partition
  (contraction) dim. The 3x3 conv uses a zero-padded 16x16 per-image SBUF
  layout; each of the 9 taps is a shifted-window matmul accumulating in PSUM.
- Compute in bf16 (moving+stationary operands), fp32 PSUM accumulation,
  fp32 output. Residual is added from the bf16 x tiles on the VectorEngine;
  bias+ReLU on the ScalarEngine during PSUM eviction.
"""

import sys

if "/opt/trn_rl_repo" not in sys.path:
    sys.path.insert(0, "/opt/trn_rl_repo")

import numpy as np
import ml_dtypes

import concourse.bass as bass
import concourse.bacc as bacc
import concourse.tile as tile
from concourse import mybir
from concourse.bass_utils import run_bass_kernel_spmd

EPS = 1e-5
NCORES = 8
NLOC = 16          # images per core
C_IN = 1024
WIDTH = 256
C_OUT = 1024
HW = 196           # 14*14
PADHW = 256        # 16*16 zero-padded image
P = 128
KB1 = C_IN // P    # 8 k-blocks for conv1 / residual channel blocks
KB2 = WIDTH // P   # 2 k-blocks for conv2/conv3 input
MB3 = C_OUT // P   # 8 m-blocks for conv3 output
NPAIRS = NLOC // 2  # 8 image pairs; N=392 per matmul
NF = 2 * HW        # 392

BF16 = mybir.dt.bfloat16
F32 = mybir.dt.float32
Relu = mybir.ActivationFunctionType.Relu

_cached = {}


def _build():
    """Build + compile the SPMD NEFF (one core's program). Cached."""
    if "nc" in _cached:
        return _cached["nc"]

    nc = bacc.Bacc("TRN2", target_bir_lowering=False, debug=False,
                   num_devices=NCORES)

    xt_d = nc.dram_tensor("xt", [KB1, P, NLOC * HW], BF16, kind="ExternalInput")
    w1_d = nc.dram_tensor("w1t", [KB1, P, WIDTH], BF16, kind="ExternalInput")
    w2_d = nc.dram_tensor("w2t", [9, KB2, P, WIDTH], BF16, kind="ExternalInput")
    w3_d = nc.dram_tensor("w3t", [KB2, P, C_OUT], BF16, kind="ExternalInput")
    b1_d = nc.dram_tensor("b1", [P, KB2], F32, kind="ExternalInput")
    b2_d = nc.dram_tensor("b2", [P, KB2], F32, kind="ExternalInput")
    b3_d = nc.dram_tensor("b3", [P, MB3], F32, kind="ExternalInput")
    y_d = nc.dram_tensor("y", [MB3, P, NLOC * HW], F32, kind="ExternalOutput")

    with tile.TileContext(nc) as tc:
        _emit(tc, nc, xt_d, w1_d, w2_d, w3_d, b1_d, b2_d, b3_d, y_d)

    nc.compile()
    _cached["nc"] = nc
    return nc


def _emit(tc, nc, xt_d, w1_d, w2_d, w3_d, b1_d, b2_d, b3_d, y_d):
    import contextlib

    with contextlib.ExitStack() as ctx:
        const = ctx.enter_context(tc.tile_pool(name="const", bufs=1))
        xpool = ctx.enter_context(tc.tile_pool(name="xpool", bufs=1))
        opool = ctx.enter_context(tc.tile_pool(name="opool", bufs=1))
        psp = ctx.enter_context(tc.tile_pool(name="psp", bufs=6, space="PSUM"))
        evp = ctx.enter_context(tc.tile_pool(name="evp", bufs=4))

        # ---- Loads -------------------------------------------------------
        x_tiles = []
        for k in range(KB1):
            t = xpool.tile([P, NLOC * HW], BF16, name=f"x{k}", tag=f"x{k}")
            nc.sync.dma_start(t[:], xt_d.ap()[k])
            x_tiles.append(t)

        w1_t = []
        for k in range(KB1):
            t = const.tile([P, WIDTH], BF16, name=f"w1_{k}", tag=f"w1_{k}")
            nc.sync.dma_start(t[:], w1_d.ap()[k])
            w1_t.append(t)

        w2_t = [[None] * KB2 for _ in range(9)]
        for tap in range(9):
            for k in range(KB2):
                t = const.tile([P, WIDTH], BF16, name=f"w2_{tap}_{k}",
                               tag=f"w2_{tap}_{k}")
                nc.sync.dma_start(t[:], w2_d.ap()[tap, k])
                w2_t[tap][k] = t

        w3_t = []
        for k in range(KB2):
            t = const.tile([P, C_OUT], BF16, name=f"w3_{k}", tag=f"w3_{k}")
            nc.sync.dma_start(t[:], w3_d.ap()[k])
            w3_t.append(t)

        b1_t = const.tile([P, KB2], F32, name="b1_t", tag="b1_t")
        nc.sync.dma_start(b1_t[:], b1_d.ap())
        b2_t = const.tile([P, KB2], F32, name="b2_t", tag="b2_t")
        nc.sync.dma_start(b2_t[:], b2_d.ap())
        b3_t = const.tile([P, MB3], F32, name="b3_t", tag="b3_t")
        nc.sync.dma_start(b3_t[:], b3_d.ap())

        # Zero-padded conv1 output: per image a 16x16 field, payload at
        # rows/cols 1..14. Layout [P, NLOC*256].
        out1 = []
        for m in range(KB2):
            t = opool.tile([P, NLOC * PADHW], BF16, name=f"out1_{m}",
                           tag=f"out1_{m}")
            nc.vector.memset(t[:], 0.0)
            out1.append(t)

        out2 = []
        for m in range(KB2):
            t = opool.tile([P, NLOC * HW], BF16, name=f"out2_{m}",
                           tag=f"out2_{m}")
            out2.append(t)

        # ---- conv1 (1x1, 1024->256) + bias + relu -> padded out1 --------
        for np_ in range(NPAIRS):
            for m in range(KB2):
                ps = psp.tile([P, NF], F32, name="ps1", tag="ps")
                for k in range(KB1):
                    nc.tensor.matmul(
                        ps[:],
                        w1_t[k][:, m * P:(m + 1) * P],
                        x_tiles[k][:, np_ * NF:(np_ + 1) * NF],
                        start=(k == 0), stop=(k == KB1 - 1),
                    )
                dst = (out1[m][:, np_ * 2 * PADHW:(np_ + 1) * 2 * PADHW]
                       .rearrange("p (i r c) -> p i r c", i=2, r=16, c=16)
                       [:, :, 1:15, 1:15])
                src = ps[:].rearrange("p (i r c) -> p i r c", i=2, r=14, c=14)
                nc.scalar.activation(dst, src, Relu, bias=b1_t[:, m:m + 1])

        # ---- conv2 (3x3, 256->256, pad 1) + bias + relu -> out2 ----------
        for np_ in range(NPAIRS):
            pads = [
                out1[k][:, np_ * 2 * PADHW:(np_ + 1) * 2 * PADHW]
                .rearrange("p (i r c) -> p i r c", i=2, r=16, c=16)
                for k in range(KB2)
            ]
            for m in range(KB2):
                ps = psp.tile([P, NF], F32, name="ps2", tag="ps")
                idx = 0
                for k in range(KB2):
                    for dy in range(3):
                        for dx in range(3):
                            rhs = pads[k][:, :, dy:dy + 14, dx:dx + 14]
                            nc.tensor.matmul(
                                ps[:].rearrange("p (i r c) -> p i r c",
                                                i=2, r=14, c=14),
                                w2_t[dy * 3 + dx][k][:, m * P:(m + 1) * P],
                                rhs,
                                start=(idx == 0), stop=(idx == 17),
                            )
                            idx += 1
                nc.scalar.activation(out2[m][:, np_ * NF:(np_ + 1) * NF],
                                     ps[:], Relu, bias=b2_t[:, m:m + 1])

        # ---- conv3 (1x1, 256->1024) + bias + residual + relu -> y --------
        for np_ in range(NPAIRS):
            for m in range(MB3):
                ps = psp.tile([P, NF], F32, name="ps3", tag="ps")
                for k in range(KB2):
                    nc.tensor.matmul(
                        ps[:],
                        w3_t[k][:, m * P:(m + 1) * P],
                        out2[k][:, np_ * NF:(np_ + 1) * NF],
                        start=(k == 0), stop=(k == KB2 - 1),
                    )
                tsum = evp.tile([P, NF], F32, name="tsum", tag="tsum")
                nc.vector.tensor_add(tsum[:], ps[:],
                                     x_tiles[m][:, np_ * NF:(np_ + 1) * NF])
                yt = evp.tile([P, NF], F32, name="yt", tag="yt")
                nc.scalar.activation(yt[:], tsum[:], Relu,
                                     bias=b3_t[:, m:m + 1])
                nc.sync.dma_start(
                    y_d.ap()[m][:, np_ * NF:(np_ + 1) * NF], yt[:])


def _prep(x, w1, g1, b1, m1, v1, w2, g2, b2, m2, v2, w3, g3, b3, m3, v3):
    """Host-side: fold BN, transpose weights to lhsT layouts, shard x."""
    def fold(w, g, b, m, v):
        scale = (g.astype(np.float64) / np.sqrt(v.astype(np.float64) + EPS))
        bias = b.astype(np.float64) - m.astype(np.float64) * scale
        wf = w.astype(np.float64) * scale.reshape(-1, *([1] * (w.ndim - 1)))
        return wf.astype(np.float32), bias.astype(np.float32)

    w1f, bias1 = fold(w1, g1, b1, m1, v1)   # [256,1024,1,1]
    w2f, bias2 = fold(w2, g2, b2, m2, v2)   # [256,256,3,3]
    w3f, bias3 = fold(w3, g3, b3, m3, v3)   # [1024,256,1,1]

    bf = ml_dtypes.bfloat16
    # lhsT layouts: [kblock, P(=ci), co]
    w1t = (w1f[:, :, 0, 0].T.reshape(KB1, P, WIDTH)).astype(bf)
    # [tap, kblock, P(=ci), co], tap = dy*3+dx
    w2t = (w2f.transpose(2, 3, 1, 0).reshape(9, KB2, P, WIDTH)).astype(bf)
    w3t = (w3f[:, :, 0, 0].T.reshape(KB2, P, C_OUT)).astype(bf)

    b1h = np.ascontiguousarray(bias1.reshape(KB2, P).T)   # [P, 2]
    b2h = np.ascontiguousarray(bias2.reshape(KB2, P).T)   # [P, 2]
    b3h = np.ascontiguousarray(bias3.reshape(MB3, P).T)   # [P, 8]

    # x: [128, 1024, 14, 14] -> per core [KB1, P, NLOC*HW] bf16
    xs = (x.reshape(NCORES, NLOC, KB1, P, HW)
          .transpose(0, 2, 3, 1, 4)
          .reshape(NCORES, KB1, P, NLOC * HW)).astype(bf)

    common = {"w1t": w1t, "w2t": w2t, "w3t": w3t,
              "b1": b1h, "b2": b2h, "b3": b3h}
    in_maps = [dict(common, xt=np.ascontiguousarray(xs[i]))
               for i in range(NCORES)]
    return in_maps


def kernel(**inputs):
    x = inputs["x"]
    in_maps = _prep(**inputs)
    nc = _build()
    res = run_bass_kernel_spmd(nc, in_maps, core_ids=list(range(NCORES)))

    y = np.empty((NCORES * NLOC, C_OUT, 14, 14), dtype=np.float32)
    for i in range(NCORES):
        r = res.results[i]["y"]                 # [MB3, P, NLOC*HW]
        r = (r.reshape(MB3, P, NLOC, HW)
             .transpose(2, 0, 1, 3)
             .reshape(NLOC, C_OUT, 14, 14))
        y[i * NLOC:(i + 1) * NLOC] = r
    return y
